# revision 23
# baseline (speedup 1.0000x reference)
"""CartBasisStressHead kernel for Trainium2 (8 NeuronCores, SPMD data-parallel).

Strategy
--------
Only 6 of the 9 m-rows of node_embedding are used: row 0 feeds a SiLU MLP
(per-node scalar), rows 4:9 feed a per-channel contraction (l=2 branch).
Nodes are sharded contiguously across 8 cores; segment sums are linear, so
the host combines partials for graphs that straddle tile/shard boundaries.

The steady state is bound by the scalar (ACT) engine: two [128,1024] SiLU
passes per 1024-node group at ~1 col/cycle are ~2.05us/group, and every
other engine is scheduled to hide underneath that. Key choices:

* Everything rides the wire in fp8-e4m3; W1 is decomposed hi+lo and
  contracted in one DoubleRow (double-pumped fp8) matmul.
* w_l2 is folded into the l=2 stream on the host, and within-graph node
  PAIRS and channel PAIRS are pre-summed exactly in f32 (<2% of model
  FLOPs) before error-feedback fp8 quantization, so the l=2 wire cost is
  160B/node and per-graph sums stay good to ~1 ulp.
* The segment-sum matmuls (0/1 indicator A stationary, el2w moving, one
  per 128-pair k-tile) and the four 256-col w3 chunks are spread across
  the four PE column quadrants and run concurrently.
* PSUM: h1 double-buffered (4 banks) + h2 (2) + w3 (1) + aniso S (1)
  fills all 8 banks; the aniso c-reduce runs straight out of PSUM on the
  vector engine into a staged SBUF tile.
* The scalar queue carries nothing but the 50 activations (plus a 1-col
  dummy that pulls the SiLU table load into the DMA fill window); consts
  and outputs ride the sync queue, gpsimd stays empty (its SWDGE drain
  costs ~3us of teardown otherwise).
* Dummy matmuls on zeroed scratch warm the PE's HAM clock gate during the
  DMA fill phase and keep it open through the first groups, so real
  matmuls run at 2.4 GHz from the start.

Host epilogue: bincount segment-sum of per-node scalars, scatter-add of
per-(group,ktile) aniso partials, and the tiny (G,9)@(9,9) basis change.
"""

import sys

if "/opt/trn_rl_repo" not in sys.path:
    sys.path.insert(0, "/opt/trn_rl_repo")

import numpy as np
import ml_dtypes

import concourse.bacc as bacc
import concourse.tile as tile
from concourse import mybir
from concourse import bass_utils

_S2 = 2.0 ** -0.5
_S3 = 3.0 ** -0.5
_S6 = 6.0 ** -0.5
_CG = np.array([
    [_S3, 0, 0, 0, _S3, 0, 0, 0, _S3],
    [0, 0, 0, 0, 0, _S2, 0, -_S2, 0],
    [0, 0, -_S2, 0, 0, 0, _S2, 0, 0],
    [0, _S2, 0, -_S2, 0, 0, 0, 0, 0],
    [0, 0, _S2, 0, 0, 0, _S2, 0, 0],
    [0, 0, 0, 0, 0, _S2, 0, _S2, 0],
    [-_S6, 0, 0, 0, 2 * _S6, 0, 0, 0, -_S6],
    [0, _S2, 0, _S2, 0, 0, 0, 0, 0],
    [-_S2, 0, 0, 0, 0, 0, 0, 0, _S2],
], dtype=np.float32)  # (9, 9)

N_CORES = 8
P = 128          # SBUF partitions
NG = 1024        # nodes per MLP group
PG = 512         # pair-rows per aniso group (covers NG nodes when even)
KT = PG // P     # k-tiles per aniso group (4)
ML2 = 5 * 64     # 320 l=2 values per pair-row (c pre-paired on host)
OB = 2           # groups per output staging batch

F32 = mybir.dt.float32
BF16 = mybir.dt.bfloat16
FP8 = mybir.dt.float8e4
WIRE8 = ml_dtypes.float8_e4m3
WIRE16 = ml_dtypes.bfloat16

_BUILD_CACHE = {}


def _build(n_groups, W, n_real, n_pair):
    """One build per (group count, indicator width, real node count, real
    pair count). MLP stages bound by n_real, aniso stages by n_pair."""
    key = (n_groups, W, n_real, n_pair)
    if key in _BUILD_CACHE:
        return _BUILD_CACHE[key]

    n_pad = n_groups * NG
    n_ob = (n_groups + OB - 1) // OB
    dr = mybir.MatmulPerfMode.DoubleRow
    silu = mybir.ActivationFunctionType.Silu

    nc = bacc.Bacc("TRN2", target_bir_lowering=False, debug=False,
                   num_devices=N_CORES)

    x0T = nc.dram_tensor("x0T", (P, n_pad), FP8, kind="ExternalInput").ap()
    # host pre-tiled pair stream: [group, p, (kt, mflat)] per partition
    embL2 = nc.dram_tensor("embL2", (n_groups, P, KT * ML2), FP8,
                           kind="ExternalInput").ap()
    # host-built indicator: [p, (group, kt, W)]
    A_in = nc.dram_tensor("A_in", (P, n_groups * KT * W), FP8,
                          kind="ExternalInput").ap()
    w1hl = nc.dram_tensor("w1hl", (P, 2 * P), FP8, kind="ExternalInput").ap()
    w2t = nc.dram_tensor("w2t", (P, P), BF16, kind="ExternalInput").ap()
    w3t = nc.dram_tensor("w3t", (P, 1), BF16, kind="ExternalInput").ap()
    b1 = nc.dram_tensor("b1c", (P, 1), F32, kind="ExternalInput").ap()
    b2 = nc.dram_tensor("b2c", (P, 1), F32, kind="ExternalInput").ap()
    scal = nc.dram_tensor("scal", (n_ob, 4, OB * 256), F32,
                          kind="ExternalOutput").ap()
    S_out = nc.dram_tensor("S_out", (n_ob, P, OB * 5), F32,
                           kind="ExternalOutput").ap()

    with tile.TileContext(nc) as tc:
        with (
            tc.tile_pool(name="const", bufs=1) as cpool,
            tc.tile_pool(name="x0p", bufs=6) as x0p,
            tc.tile_pool(name="el2p", bufs=6) as el2p,
            tc.tile_pool(name="hp", bufs=6) as hp,
            tc.tile_pool(name="stp", bufs=3) as stp,
            tc.tile_pool(name="ph1", bufs=2, space="PSUM") as ph1p,
            tc.tile_pool(name="ph2", bufs=1, space="PSUM") as ph2p,
            tc.tile_pool(name="psc", bufs=1, space="PSUM") as pscp,
            tc.tile_pool(name="pS", bufs=1, space="PSUM") as pSp,
        ):
            w1s = cpool.tile([P, 2 * P], FP8)
            w2s = cpool.tile([P, P], BF16)
            w3s = cpool.tile([P, 1], BF16)
            b1s = cpool.tile([P, 1], F32)
            b2s = cpool.tile([P, 1], F32)
            Aall = cpool.tile([P, n_groups * KT * W], FP8)
            nc.sync.dma_start(out=w1s[:], in_=w1hl)
            nc.sync.dma_start(out=b1s[:], in_=b1)
            warm = cpool.tile([P, 1], BF16)
            nc.scalar.activation(warm[:], b1s[:],
                                 mybir.ActivationFunctionType.Silu)
            # PE warmup: dummy matmuls on zeroed scratch while input DMAs
            # are in flight, so the HAM clock-gate releases (1.2 -> 2.4
            # GHz) before the first real matmul.
            dumW = cpool.tile([P, P], BF16)
            dumM = cpool.tile([P, 512], BF16)
            nc.vector.memset(dumW[:], 0.0)
            nc.vector.memset(dumM[:], 0.0)
            dum = ph1p.tile([P, NG], F32, tag="h1p")
            for _ in range(8):
                nc.tensor.matmul(dum[:, 0:512], dumW[:], dumM[:],
                                 start=True, stop=True)

            # Cross-group software pipeline: L2 lags one group, w3 two, so
            # the in-order Tensor queue never waits on a SiLU.
            h1l = {}   # g -> h1s
            h2l = {}   # g -> h2s
            scl = {}   # g -> scp psum tile
            srl = {}   # g -> Sr (512-col chunks of real nodes)
            scst = Sst = None
            scst_by_g = {}

            x0l = {}   # g -> x0c tile
            el2l = {}  # g -> el2c tile
            pend = []  # deferred output DMAs: (dram_slice, sbuf_ap)

            def stage_in(g, has_mlp, pr_live):
                if has_mlp:
                    grp_real = min(NG, n_real - g * NG)
                    Sr = (grp_real + 511) // 512
                    srl[g] = Sr
                    x0c = x0p.tile([P, NG], FP8, tag="x0c")
                    nc.sync.dma_start(
                        out=x0c[:, :Sr * 512],
                        in_=x0T[:, g * NG: g * NG + Sr * 512])
                    x0l[g] = x0c
                if pr_live > 0:
                    el2c = el2p.tile([P, KT * ML2], FP8, tag="el2c")
                    Kl = (pr_live + P - 1) // P
                    nc.sync.dma_start(
                        out=el2c[:, :Kl * ML2],
                        in_=embL2[g][:, :Kl * ML2])
                    el2l[g] = el2c


            def stage_l1(g):
                Sr = srl[g]
                x0c = x0l.pop(g)
                h1p = ph1p.tile([P, NG], F32, tag="h1p")
                for s in range(Sr):
                    nsl = slice(s * 512, (s + 1) * 512)
                    nc.tensor.matmul(
                        h1p[:, nsl],
                        w1s[:].rearrange("p (k h) -> p k h", k=2, h=P),
                        x0c[:, nsl].unsqueeze(1).to_broadcast([P, 2, 512]),
                        start=True, stop=True, perf_mode=dr)
                h1s = hp.tile([P, NG], BF16, tag="h1s")
                nc.scalar.activation(h1s[:, :Sr * 512], h1p[:, :Sr * 512],
                                     silu, bias=b1s[:])
                h1l[g] = h1s

            def stage_l2(g):
                Sr = srl[g]
                h2p = ph2p.tile([P, NG], F32, tag="h2p")
                for s in range(Sr):
                    nsl = slice(s * 512, (s + 1) * 512)
                    nc.tensor.matmul(h2p[:, nsl], w2s[:], h1l[g][:, nsl],
                                     start=True, stop=True)
                h2s = hp.tile([P, NG], BF16, tag="h2s")
                nc.scalar.activation(h2s[:, :Sr * 512], h2p[:, :Sr * 512],
                                     silu, bias=b2s[:])
                h2l[g] = h2s
                del h1l[g]

            def stage_w3(g):
                # 4 chunks of 256 nodes, one per PE column quadrant; chunk c
                # lands on psum partition 32c, cols 0:256 of one bank.
                nonlocal scst
                grp_real = min(NG, n_real - g * NG)
                Cl = (grp_real + 255) // 256
                if g % OB == 0:
                    scst = stp.tile([97, OB * 256], F32, tag="scst")
                scst_by_g[g] = scst
                scp = pscp.tile([P, 256], F32, tag="scp")
                scl[g] = scp
                for c in range(Cl):
                    q = 32 * c
                    nc.tensor.matmul(scp[q:q + 1, :],
                                     w3s[:],
                                     h2l[g][:, c * 256:(c + 1) * 256],
                                     start=True, stop=True,
                                     tile_position=(0, q))
                del h2l[g]

            def stage_scst(g):
                boff = g % OB
                scp = scl.pop(g)
                nc.vector.tensor_copy(
                    out=scst_by_g[g][:, boff * 256:(boff + 1) * 256],
                    in_=scp[0:97, :])
                if g % OB == OB - 1 or g == n_groups - 1:
                    pend.append((scal[g // OB],
                                 scst_by_g[g][0:97:32, :]))
                del scst_by_g[g]

            for grp in range(n_groups):
                has_mlp = grp * NG < n_real
                pr_live = min(PG, n_pair - grp * PG)
                stage_in(grp, has_mlp, pr_live)
                while pend:
                    o, i = pend.pop(0)
                    nc.sync.dma_start(out=o, in_=i)
                if grp == 0:
                    nc.sync.dma_start(out=Aall[:], in_=A_in)
                    nc.sync.dma_start(out=w2s[:], in_=w2t)
                    nc.sync.dma_start(out=b2s[:], in_=b2)
                    nc.sync.dma_start(out=w3s[:], in_=w3t)
                if has_mlp:
                    stage_l1(grp)
                if grp >= 1 and (grp - 1) in h1l:
                    stage_l2(grp - 1)
                if grp >= 2 and (grp - 2) in h2l:
                    stage_w3(grp - 2)
                    stage_scst(grp - 2)

                # ---- l=2 branch: fp8 segment-sum matmuls, k-tile t on
                # column quadrant 32t, psum rows 32t..32t+W ----
                if pr_live > 0:
                    el2c = el2l.pop(grp)
                    Kl = (pr_live + P - 1) // P
                    if grp % OB == 0:
                        Sst = stp.tile([P, OB * 5], F32, tag="SstR")
                    boff = grp % OB
                    pS4 = pSp.tile([P, 512], F32, tag="pS4")
                    Abase = grp * KT * W
                    for t in range(Kl):
                        As = Aall[:, Abase + t * W: Abase + (t + 1) * W]
                        ro = 32 * (t % 4)
                        nc.tensor.matmul(pS4[ro:ro + W, 0:ML2], As,
                                         el2c[:, t * ML2:(t + 1) * ML2],
                                         start=(t < 4), stop=(t + 4 >= Kl),
                                         tile_position=(0, ro))
                    # c-chunk reduce straight out of PSUM:
                    # [p, 5(m), 64(c-pairs)] -> sum over 64
                    nc.vector.tensor_reduce(
                        out=Sst[:, boff * 5:(boff + 1) * 5],
                        in_=pS4[:, 0:ML2]
                        .rearrange("p (f c) -> p f c", f=5, c=64),
                        axis=mybir.AxisListType.X,
                        op=mybir.AluOpType.add)
                    if grp % OB == OB - 1 or grp == n_groups - 1:
                        pend.append((S_out[grp // OB], Sst[:]))
                    if grp < 5:
                        # keep the PE's HAM clock-gate open through the
                        # DMA-limited fill phase
                        dumg = pSp.tile([P, 512], F32, tag="pS4")
                        for _ in range(2):
                            nc.tensor.matmul(dumg[:, :], dumW[:], dumM[:],
                                             start=True, stop=True)

            # pipeline flush
            if (n_groups - 1) in h1l:
                stage_l2(n_groups - 1)
            for g in (n_groups - 2, n_groups - 1):
                if g >= 0 and g in h2l:
                    stage_w3(g)
                    stage_scst(g)
            while pend:
                o, i = pend.pop(0)
                nc.sync.dma_start(out=o, in_=i)
    nc.compile()
    _BUILD_CACHE[key] = nc
    return nc


def _fb_quant_groups(x, span):
    """Error-feedback fp8 quantization along axis 1 (pair-in-group axis).

    x: (n_fibers, span, feat) float32. The carry telescopes rounding error
    along each group's scan so segment sums of the quantized values track
    the exact sums to ~1 ulp.
    """
    q = np.empty(x.shape, WIRE8)
    carry = np.zeros((x.shape[0], x.shape[2]), np.float32)
    for i in range(span):
        carry += x[:, i, :]
        qi = carry.astype(WIRE8)
        q[:, i, :] = qi
        carry -= qi.astype(np.float32)
    return q


def _host_reference(node_embedding, W1, b1, W2, b2, W3, b3, w_l2, batch,
                    natoms):
    """Pure-numpy fallback (only used for pathological graph layouts)."""
    G = natoms.shape[0]
    inv = 1.0 / natoms.astype(np.float32)
    x = node_embedding[:, 0, :]
    h = x @ W1.T + b1
    h = h / (1.0 + np.exp(-h))
    h = h @ W2.T + b2
    h = h / (1.0 + np.exp(-h))
    ns = (h @ W3.T + b3)[:, 0]
    ok = (batch >= 0) & (batch < G)
    bok = batch[ok]
    iso = np.bincount(bok, weights=ns[ok], minlength=G).astype(np.float32) \
        * inv
    nl2 = np.einsum("nmc,c->nm", node_embedding[:, 4:9, :], w_l2[0])
    aniso = np.stack(
        [np.bincount(bok, weights=nl2[ok, m], minlength=G)
         for m in range(5)], axis=1).astype(np.float32) * inv[:, None]
    dec = np.concatenate([iso[:, None], np.zeros((G, 3), np.float32), aniso],
                         axis=1)
    return (dec @ _CG).reshape(-1, 3, 3).astype(np.float32)


def kernel(node_embedding, W1, b1, W2, b2, W3, b3, w_l2, batch, natoms):
    node_embedding = np.asarray(node_embedding, dtype=np.float32)
    W1 = np.asarray(W1, dtype=np.float32)
    b1 = np.asarray(b1, dtype=np.float32)
    W2 = np.asarray(W2, dtype=np.float32)
    b2 = np.asarray(b2, dtype=np.float32)
    W3 = np.asarray(W3, dtype=np.float32)
    b3 = np.asarray(b3, dtype=np.float32)
    w_l2 = np.asarray(w_l2, dtype=np.float32)
    batch = np.asarray(batch).astype(np.int64)
    natoms_in = np.asarray(natoms)

    N = node_embedding.shape[0]
    G = natoms_in.shape[0]
    if (not np.all(batch[:-1] <= batch[1:])
            or batch.min(initial=0) < 0 or batch.max(initial=0) >= G):
        return _host_reference(node_embedding, W1, b1, W2, b2, W3, b3,
                               w_l2, batch, natoms_in)

    n_sh = (N + N_CORES - 1) // N_CORES
    n_groups = (n_sh + NG - 1) // NG
    n_pad = n_groups * NG
    n_ppad = n_groups * PG

    # ---- per-core shards; per-core pairing of consecutive same-graph
    # nodes (host pre-sums pairs exactly in f32) ----
    shards = []
    W_need = 1
    ok = True
    for c in range(N_CORES):
        n0 = min(c * n_sh, N)
        n1 = min(n0 + n_sh, N)
        b = batch[n0:n1]
        nreal = n1 - n0
        if nreal == 0:
            shards.append((n0, n1, b, None, None,
                           np.zeros(n_groups, np.int64), 0))
            continue
        chg = np.flatnonzero(np.diff(b)) + 1             # run starts (mid)
        starts = np.concatenate([[0], chg])
        run_id = np.zeros(nreal, np.int64)
        run_id[chg] = 1
        run_id = np.cumsum(run_id)                       # run per node
        idx_in_run = np.arange(nreal) - starts[run_id]
        run_len = np.diff(np.concatenate([starts, [nreal]]))
        run_pairs = (run_len + 1) // 2
        pairbase = np.concatenate([[0], np.cumsum(run_pairs)[:-1]])
        pr = pairbase[run_id] + (idx_in_run >> 1)        # pair-row per node
        npair = int(run_pairs.sum())
        if npair > n_ppad:
            ok = False
            break
        pair_batch = np.zeros(npair, np.int64)
        pair_batch[pr] = b
        gbase = np.zeros(n_groups, np.int64)
        for grp in range(n_groups):
            lo = grp * PG
            hi = min(lo + PG, npair)
            if lo < npair:
                gbase[grp] = pair_batch[lo]
                span = int(pair_batch[hi - 1] - pair_batch[lo] + 1)
                W_need = max(W_need, span)
        shards.append((n0, n1, b, pr, pair_batch, gbase, npair))
    if not ok or W_need > 32:
        return _host_reference(node_embedding, W1, b1, W2, b2, W3, b3,
                               w_l2, batch, natoms_in)
    W = 16 if W_need <= 16 else 32

    W1hi = W1.astype(WIRE8).astype(np.float32)
    W1lo = (W1 - W1hi).astype(WIRE8)
    w1hl = np.ascontiguousarray(
        np.stack([W1hi.astype(WIRE8).T, W1lo.T], axis=1)
        .reshape(P, 2 * P)).astype(WIRE8)
    w2t = np.ascontiguousarray(W2.T).astype(WIRE16)
    w3t = np.ascontiguousarray(W3.T).astype(WIRE16)
    b1c = np.ascontiguousarray(b1[:, None])
    b2c = np.ascontiguousarray(b2[:, None])

    # ---- l=2 stream: fold w, pre-sum node pairs and c pairs,
    # fb-quantize ----
    el2w = (node_embedding[:, 4:9, :] * w_l2[0][None, None, :]) \
        .reshape(N, 5, 64, 2).sum(-1).reshape(N, ML2)
    el2_all = np.zeros((N_CORES, n_ppad, ML2), np.float32)
    for c in range(N_CORES):
        n0, n1, b, pr, pair_batch, gbase, npair = shards[c]
        if npair == 0:
            continue
        seg = el2w[n0:n1]
        if (n1 - n0) % 2 == 0 and np.array_equal(b[0::2], b[1::2]):
            el2_all[c, :npair] = seg[0::2] + seg[1::2]
        else:
            np.add.at(el2_all[c], pr, seg)
    el2q = _fb_quant_groups(
        el2_all.reshape(N_CORES * n_groups, PG, ML2), PG) \
        .reshape(N_CORES, n_ppad, ML2)

    in_maps = []
    for c in range(N_CORES):
        n0, n1, b, pr, pair_batch, gbase, npair = shards[c]
        nreal = n1 - n0
        x0T = np.zeros((P, n_pad), WIRE8)
        x0T[:, :nreal] = node_embedding[n0:n1, 0, :].T.astype(WIRE8)
        # pair-row = grp*PG + kt*P + p
        el2 = np.ascontiguousarray(
            el2q[c].reshape(n_groups, KT, P, ML2)
            .transpose(0, 2, 1, 3).reshape(n_groups, P, KT * ML2))
        lg = np.full(n_ppad, -1.0, np.float32)
        if npair:
            lg[:npair] = (pair_batch
                          - np.repeat(gbase, PG)[:npair]).astype(np.float32)
        A = (lg.reshape(n_groups, KT, P)[..., None]
             == np.arange(W, dtype=np.float32)).astype(WIRE8)
        A = np.ascontiguousarray(
            A.transpose(2, 0, 1, 3).reshape(P, n_groups * KT * W))
        in_maps.append({
            "x0T": x0T, "embL2": el2, "A_in": A,
            "w1hl": w1hl, "w2t": w2t, "w3t": w3t, "b1c": b1c, "b2c": b2c,
        })

    # all cores share shapes for the common layouts; build on core 0's
    nreal0 = shards[0][1] - shards[0][0]
    npair0 = shards[0][6]
    same = all((s[1] - s[0], s[6]) == (nreal0, npair0) for s in shards)
    if not same:
        # distinct per-core shapes: build with max bounds (padded inputs
        # make the extra work read zeros)
        nreal0 = max(s[1] - s[0] for s in shards)
        npair0 = max(s[6] for s in shards)
    nc = _build(n_groups, W, nreal0, npair0)

    res = bass_utils.run_bass_kernel_spmd(nc, in_maps,
                                          core_ids=list(range(N_CORES)))

    # ---- host epilogue ----
    inv = (1.0 / natoms_in.astype(np.float32)).astype(np.float32)
    n_ob = (n_groups + OB - 1) // OB
    node_scalar = np.empty(N, np.float32)
    Afull = np.zeros((G + 64, 5), np.float32)
    for c in range(N_CORES):
        n0, n1, b, pr, pair_batch, gbase, npair = shards[c]
        nreal = n1 - n0
        if nreal == 0:
            continue
        # scal layout: [n_ob, 4(chunk), OB, 256]; node = g*NG + c*256 + j
        sc = res.results[c]["scal"].reshape(n_ob, 4, OB, 256) \
            .transpose(0, 2, 1, 3).reshape(-1)[:nreal]
        node_scalar[n0:n1] = sc
        # S layout: [n_ob, 128, OB*5]; k-tile t -> rows 32t..32t+W
        Sc = res.results[c]["S_out"]
        for grp in range(n_groups):
            if grp * PG >= npair:
                continue
            gb = int(gbase[grp])
            j = grp % OB
            blk = Sc[grp // OB][:, j * 5:(j + 1) * 5]    # (128, 5)
            live = min(PG, npair - grp * PG)
            Kl = (live + P - 1) // P
            av = np.zeros((W, 5), np.float32)
            for t in range(min(Kl, 4)):
                av += blk[32 * t: 32 * t + W]
            Afull[gb:gb + W] += av
    iso = np.bincount(batch, weights=node_scalar + b3[0], minlength=G)
    iso = iso.astype(np.float32) * inv
    aniso = Afull[:G] * inv[:, None]
    dec = np.concatenate([iso[:, None], np.zeros((G, 3), np.float32), aniso],
                         axis=1)
    return (dec @ _CG).reshape(-1, 3, 3).astype(np.float32)


# revision 24
# speedup vs baseline: 1.0184x; 1.0184x over previous
"""CartBasisStressHead kernel for Trainium2 (8 NeuronCores, SPMD data-parallel).

Strategy
--------
Only 6 of the 9 m-rows of node_embedding are used: row 0 feeds a SiLU MLP
(per-node scalar), rows 4:9 feed a per-channel contraction (l=2 branch).
Nodes are sharded contiguously across 8 cores; segment sums are linear, so
the host combines partials for graphs that straddle tile/shard boundaries.

The steady state is bound by the scalar (ACT) engine: two [128,1024] SiLU
passes per 1024-node group at ~1 col/cycle are ~2.05us/group, and every
other engine is scheduled to hide underneath that. Key choices:

* Everything rides the wire in fp8-e4m3; W1 is decomposed hi+lo and
  contracted in one DoubleRow (double-pumped fp8) matmul.
* w_l2 is folded into the l=2 stream on the host, and within-graph node
  PAIRS and channel PAIRS are pre-summed exactly in f32 (<2% of model
  FLOPs) before error-feedback fp8 quantization, so the l=2 wire cost is
  160B/node and per-graph sums stay good to ~1 ulp.
* The segment-sum matmuls (0/1 indicator A stationary, el2w moving, one
  per 128-pair k-tile) and the four 256-col w3 chunks are spread across
  the four PE column quadrants and run concurrently.
* PSUM: h1 double-buffered (4 banks) + h2 (2) + w3 (1) + aniso S (1)
  fills all 8 banks; the aniso c-reduce runs straight out of PSUM on the
  vector engine into a staged SBUF tile.
* The scalar queue carries nothing but the 50 activations (plus a 1-col
  dummy that pulls the SiLU table load into the DMA fill window); consts
  and outputs ride the sync queue, gpsimd stays empty (its SWDGE drain
  costs ~3us of teardown otherwise).
* Dummy matmuls on zeroed scratch warm the PE's HAM clock gate during the
  DMA fill phase and keep it open through the first groups, so real
  matmuls run at 2.4 GHz from the start.

Host epilogue: bincount segment-sum of per-node scalars, scatter-add of
per-(group,ktile) aniso partials, and the tiny (G,9)@(9,9) basis change.
"""

import sys

if "/opt/trn_rl_repo" not in sys.path:
    sys.path.insert(0, "/opt/trn_rl_repo")

import numpy as np
import ml_dtypes

import concourse.bacc as bacc
import concourse.tile as tile
from concourse import mybir
from concourse import bass_utils

_S2 = 2.0 ** -0.5
_S3 = 3.0 ** -0.5
_S6 = 6.0 ** -0.5
_CG = np.array([
    [_S3, 0, 0, 0, _S3, 0, 0, 0, _S3],
    [0, 0, 0, 0, 0, _S2, 0, -_S2, 0],
    [0, 0, -_S2, 0, 0, 0, _S2, 0, 0],
    [0, _S2, 0, -_S2, 0, 0, 0, 0, 0],
    [0, 0, _S2, 0, 0, 0, _S2, 0, 0],
    [0, 0, 0, 0, 0, _S2, 0, _S2, 0],
    [-_S6, 0, 0, 0, 2 * _S6, 0, 0, 0, -_S6],
    [0, _S2, 0, _S2, 0, 0, 0, 0, 0],
    [-_S2, 0, 0, 0, 0, 0, 0, 0, _S2],
], dtype=np.float32)  # (9, 9)

N_CORES = 8
P = 128          # SBUF partitions
NG = 1024        # nodes per MLP group
PG = 512         # pair-rows per aniso group (covers NG nodes when even)
KT = PG // P     # k-tiles per aniso group (4)
ML2 = 5 * 64     # 320 l=2 values per pair-row (c pre-paired on host)
OB = 2           # groups per output staging batch

F32 = mybir.dt.float32
BF16 = mybir.dt.bfloat16
FP8 = mybir.dt.float8e4
WIRE8 = ml_dtypes.float8_e4m3
WIRE16 = ml_dtypes.bfloat16

_BUILD_CACHE = {}


def _build(n_groups, W, n_real, n_pair):
    """One build per (group count, indicator width, real node count, real
    pair count). MLP stages bound by n_real, aniso stages by n_pair."""
    key = (n_groups, W, n_real, n_pair)
    if key in _BUILD_CACHE:
        return _BUILD_CACHE[key]

    n_pad = n_groups * NG
    n_ob = (n_groups + OB - 1) // OB
    dr = mybir.MatmulPerfMode.DoubleRow
    silu = mybir.ActivationFunctionType.Silu

    nc = bacc.Bacc("TRN2", target_bir_lowering=False, debug=False,
                   num_devices=N_CORES)

    x0T = nc.dram_tensor("x0T", (P, n_pad), FP8, kind="ExternalInput").ap()
    # host pre-tiled pair stream: [group, p, (kt, mflat)] per partition
    embL2 = nc.dram_tensor("embL2", (n_groups, P, KT * ML2), FP8,
                           kind="ExternalInput").ap()
    # host-built indicator: [p, (group, kt, W)]
    A_in = nc.dram_tensor("A_in", (P, n_groups * KT * W), FP8,
                          kind="ExternalInput").ap()
    w1hl = nc.dram_tensor("w1hl", (P, 2 * P), FP8, kind="ExternalInput").ap()
    w2t = nc.dram_tensor("w2t", (P, P), BF16, kind="ExternalInput").ap()
    w3t = nc.dram_tensor("w3t", (P, 1), BF16, kind="ExternalInput").ap()
    b1 = nc.dram_tensor("b1c", (P, 1), F32, kind="ExternalInput").ap()
    b2 = nc.dram_tensor("b2c", (P, 1), F32, kind="ExternalInput").ap()
    scal = nc.dram_tensor("scal", (n_ob, 4, OB * 256), F32,
                          kind="ExternalOutput").ap()
    S_out = nc.dram_tensor("S_out", (n_ob, P, OB * 5), F32,
                           kind="ExternalOutput").ap()

    with tile.TileContext(nc) as tc:
        with (
            tc.tile_pool(name="const", bufs=1) as cpool,
            tc.tile_pool(name="x0p", bufs=6) as x0p,
            tc.tile_pool(name="el2p", bufs=6) as el2p,
            tc.tile_pool(name="hp", bufs=6) as hp,
            tc.tile_pool(name="stp", bufs=3) as stp,
            tc.tile_pool(name="ph1", bufs=2, space="PSUM") as ph1p,
            tc.tile_pool(name="ph2", bufs=1, space="PSUM") as ph2p,
            tc.tile_pool(name="psc", bufs=1, space="PSUM") as pscp,
            tc.tile_pool(name="pS", bufs=1, space="PSUM") as pSp,
        ):
            w1s = cpool.tile([P, 2 * P], FP8)
            w2s = cpool.tile([P, P], BF16)
            w3s = cpool.tile([P, 1], BF16)
            b1s = cpool.tile([P, 1], F32)
            b2s = cpool.tile([P, 1], F32)
            Aall = cpool.tile([P, n_groups * KT * W], FP8)
            nc.sync.dma_start(out=w1s[:], in_=w1hl)
            nc.sync.dma_start(out=b1s[:], in_=b1)
            warm = cpool.tile([P, 1], BF16)
            nc.scalar.activation(warm[:], b1s[:],
                                 mybir.ActivationFunctionType.Silu)
            # PE warmup: dummy matmuls on zeroed scratch while input DMAs
            # are in flight, so the HAM clock-gate releases (1.2 -> 2.4
            # GHz) before the first real matmul.
            dumW = cpool.tile([P, P], BF16)
            dumM = cpool.tile([P, 512], BF16)
            nc.vector.memset(dumW[:], 0.0)
            nc.vector.memset(dumM[:], 0.0)
            dum = ph1p.tile([P, NG], F32, tag="h1p")
            for _ in range(8):
                nc.tensor.matmul(dum[:, 0:512], dumW[:], dumM[:],
                                 start=True, stop=True)

            # Cross-group software pipeline: L2 lags one group, w3 two, so
            # the in-order Tensor queue never waits on a SiLU.
            h1l = {}   # g -> h1s
            h2l = {}   # g -> h2s
            scl = {}   # g -> scp psum tile
            srl = {}   # g -> Sr (512-col chunks of real nodes)
            scst = Sst = None
            scst_by_g = {}

            x0l = {}   # g -> x0c tile
            el2l = {}  # g -> el2c tile

            def stage_in(g, has_mlp, pr_live):
                if has_mlp:
                    grp_real = min(NG, n_real - g * NG)
                    Sr = (grp_real + 511) // 512
                    srl[g] = Sr
                    x0c = x0p.tile([P, NG], FP8, tag="x0c")
                    nc.sync.dma_start(
                        out=x0c[:, :Sr * 512],
                        in_=x0T[:, g * NG: g * NG + Sr * 512])
                    x0l[g] = x0c
                if pr_live > 0:
                    el2c = el2p.tile([P, KT * ML2], FP8, tag="el2c")
                    Kl = (pr_live + P - 1) // P
                    nc.sync.dma_start(
                        out=el2c[:, :Kl * ML2],
                        in_=embL2[g][:, :Kl * ML2])
                    el2l[g] = el2c


            def stage_l1(g):
                Sr = srl[g]
                x0c = x0l.pop(g)
                h1p = ph1p.tile([P, NG], F32, tag="h1p")
                for s in range(Sr):
                    nsl = slice(s * 512, (s + 1) * 512)
                    nc.tensor.matmul(
                        h1p[:, nsl],
                        w1s[:].rearrange("p (k h) -> p k h", k=2, h=P),
                        x0c[:, nsl].unsqueeze(1).to_broadcast([P, 2, 512]),
                        start=True, stop=True, perf_mode=dr)
                h1s = hp.tile([P, NG], BF16, tag="h1s")
                nc.scalar.activation(h1s[:, :Sr * 512], h1p[:, :Sr * 512],
                                     silu, bias=b1s[:])
                h1l[g] = h1s

            def stage_l2(g):
                Sr = srl[g]
                h2p = ph2p.tile([P, NG], F32, tag="h2p")
                for s in range(Sr):
                    nsl = slice(s * 512, (s + 1) * 512)
                    nc.tensor.matmul(h2p[:, nsl], w2s[:], h1l[g][:, nsl],
                                     start=True, stop=True)
                h2s = hp.tile([P, NG], BF16, tag="h2s")
                nc.scalar.activation(h2s[:, :Sr * 512], h2p[:, :Sr * 512],
                                     silu, bias=b2s[:])
                h2l[g] = h2s
                del h1l[g]

            def stage_w3(g):
                # 4 chunks of 256 nodes, one per PE column quadrant; chunk c
                # lands on psum partition 32c, cols 0:256 of one bank.
                nonlocal scst
                grp_real = min(NG, n_real - g * NG)
                Cl = (grp_real + 255) // 256
                if g % OB == 0:
                    scst = stp.tile([97, OB * 256], F32, tag="scst")
                scst_by_g[g] = scst
                scp = pscp.tile([P, 256], F32, tag="scp")
                scl[g] = scp
                for c in range(Cl):
                    q = 32 * c
                    nc.tensor.matmul(scp[q:q + 1, :],
                                     w3s[:],
                                     h2l[g][:, c * 256:(c + 1) * 256],
                                     start=True, stop=True,
                                     tile_position=(0, q))
                del h2l[g]

            def stage_scst(g):
                boff = g % OB
                scp = scl.pop(g)
                nc.vector.tensor_copy(
                    out=scst_by_g[g][:, boff * 256:(boff + 1) * 256],
                    in_=scp[0:97, :])
                if g % OB == OB - 1 or g == n_groups - 1:
                    nc.sync.dma_start(out=scal[g // OB],
                                      in_=scst_by_g[g][0:97:32, :])
                del scst_by_g[g]

            for grp in range(n_groups):
                has_mlp = grp * NG < n_real
                pr_live = min(PG, n_pair - grp * PG)
                stage_in(grp, has_mlp, pr_live)
                if grp == 0:
                    nc.sync.dma_start(out=Aall[:], in_=A_in)
                    nc.sync.dma_start(out=w2s[:], in_=w2t)
                    nc.sync.dma_start(out=b2s[:], in_=b2)
                    nc.sync.dma_start(out=w3s[:], in_=w3t)
                if has_mlp:
                    stage_l1(grp)
                if grp >= 1 and (grp - 1) in h1l:
                    stage_l2(grp - 1)
                if grp >= 2 and (grp - 2) in h2l:
                    stage_w3(grp - 2)
                    stage_scst(grp - 2)

                # ---- l=2 branch: fp8 segment-sum matmuls, k-tile t on
                # column quadrant 32t, psum rows 32t..32t+W ----
                if pr_live > 0:
                    el2c = el2l.pop(grp)
                    Kl = (pr_live + P - 1) // P
                    if grp % OB == 0:
                        Sst = stp.tile([P, OB * 5], F32, tag="SstR")
                    boff = grp % OB
                    pS4 = pSp.tile([P, 512], F32, tag="pS4")
                    Abase = grp * KT * W
                    for t in range(Kl):
                        As = Aall[:, Abase + t * W: Abase + (t + 1) * W]
                        ro = 32 * (t % 4)
                        nc.tensor.matmul(pS4[ro:ro + W, 0:ML2], As,
                                         el2c[:, t * ML2:(t + 1) * ML2],
                                         start=(t < 4), stop=(t + 4 >= Kl),
                                         tile_position=(0, ro))
                    # c-chunk reduce straight out of PSUM:
                    # [p, 5(m), 64(c-pairs)] -> sum over 64
                    nc.vector.tensor_reduce(
                        out=Sst[:, boff * 5:(boff + 1) * 5],
                        in_=pS4[:, 0:ML2]
                        .rearrange("p (f c) -> p f c", f=5, c=64),
                        axis=mybir.AxisListType.X,
                        op=mybir.AluOpType.add)
                    if grp % OB == OB - 1 or grp == n_groups - 1:
                        nc.sync.dma_start(out=S_out[grp // OB],
                                          in_=Sst[:])
                    if grp < 5:
                        # keep the PE's HAM clock-gate open through the
                        # DMA-limited fill phase
                        dumg = pSp.tile([P, 512], F32, tag="pS4")
                        for _ in range(2):
                            nc.tensor.matmul(dumg[:, :], dumW[:], dumM[:],
                                             start=True, stop=True)

            # pipeline flush
            if (n_groups - 1) in h1l:
                stage_l2(n_groups - 1)
            for g in (n_groups - 2, n_groups - 1):
                if g >= 0 and g in h2l:
                    stage_w3(g)
                    stage_scst(g)
    nc.compile()
    _BUILD_CACHE[key] = nc
    return nc


def _fb_quant_groups(x, span):
    """Error-feedback fp8 quantization along axis 1 (pair-in-group axis).

    x: (n_fibers, span, feat) float32. The carry telescopes rounding error
    along each group's scan so segment sums of the quantized values track
    the exact sums to ~1 ulp.
    """
    q = np.empty(x.shape, WIRE8)
    carry = np.zeros((x.shape[0], x.shape[2]), np.float32)
    for i in range(span):
        carry += x[:, i, :]
        qi = carry.astype(WIRE8)
        q[:, i, :] = qi
        carry -= qi.astype(np.float32)
    return q


def _host_reference(node_embedding, W1, b1, W2, b2, W3, b3, w_l2, batch,
                    natoms):
    """Pure-numpy fallback (only used for pathological graph layouts)."""
    G = natoms.shape[0]
    inv = 1.0 / natoms.astype(np.float32)
    x = node_embedding[:, 0, :]
    h = x @ W1.T + b1
    h = h / (1.0 + np.exp(-h))
    h = h @ W2.T + b2
    h = h / (1.0 + np.exp(-h))
    ns = (h @ W3.T + b3)[:, 0]
    ok = (batch >= 0) & (batch < G)
    bok = batch[ok]
    iso = np.bincount(bok, weights=ns[ok], minlength=G).astype(np.float32) \
        * inv
    nl2 = np.einsum("nmc,c->nm", node_embedding[:, 4:9, :], w_l2[0])
    aniso = np.stack(
        [np.bincount(bok, weights=nl2[ok, m], minlength=G)
         for m in range(5)], axis=1).astype(np.float32) * inv[:, None]
    dec = np.concatenate([iso[:, None], np.zeros((G, 3), np.float32), aniso],
                         axis=1)
    return (dec @ _CG).reshape(-1, 3, 3).astype(np.float32)


def kernel(node_embedding, W1, b1, W2, b2, W3, b3, w_l2, batch, natoms):
    node_embedding = np.asarray(node_embedding, dtype=np.float32)
    W1 = np.asarray(W1, dtype=np.float32)
    b1 = np.asarray(b1, dtype=np.float32)
    W2 = np.asarray(W2, dtype=np.float32)
    b2 = np.asarray(b2, dtype=np.float32)
    W3 = np.asarray(W3, dtype=np.float32)
    b3 = np.asarray(b3, dtype=np.float32)
    w_l2 = np.asarray(w_l2, dtype=np.float32)
    batch = np.asarray(batch).astype(np.int64)
    natoms_in = np.asarray(natoms)

    N = node_embedding.shape[0]
    G = natoms_in.shape[0]
    if (not np.all(batch[:-1] <= batch[1:])
            or batch.min(initial=0) < 0 or batch.max(initial=0) >= G):
        return _host_reference(node_embedding, W1, b1, W2, b2, W3, b3,
                               w_l2, batch, natoms_in)

    n_sh = (N + N_CORES - 1) // N_CORES
    n_groups = (n_sh + NG - 1) // NG
    n_pad = n_groups * NG
    n_ppad = n_groups * PG

    # ---- per-core shards; per-core pairing of consecutive same-graph
    # nodes (host pre-sums pairs exactly in f32) ----
    shards = []
    W_need = 1
    ok = True
    for c in range(N_CORES):
        n0 = min(c * n_sh, N)
        n1 = min(n0 + n_sh, N)
        b = batch[n0:n1]
        nreal = n1 - n0
        if nreal == 0:
            shards.append((n0, n1, b, None, None,
                           np.zeros(n_groups, np.int64), 0))
            continue
        chg = np.flatnonzero(np.diff(b)) + 1             # run starts (mid)
        starts = np.concatenate([[0], chg])
        run_id = np.zeros(nreal, np.int64)
        run_id[chg] = 1
        run_id = np.cumsum(run_id)                       # run per node
        idx_in_run = np.arange(nreal) - starts[run_id]
        run_len = np.diff(np.concatenate([starts, [nreal]]))
        run_pairs = (run_len + 1) // 2
        pairbase = np.concatenate([[0], np.cumsum(run_pairs)[:-1]])
        pr = pairbase[run_id] + (idx_in_run >> 1)        # pair-row per node
        npair = int(run_pairs.sum())
        if npair > n_ppad:
            ok = False
            break
        pair_batch = np.zeros(npair, np.int64)
        pair_batch[pr] = b
        gbase = np.zeros(n_groups, np.int64)
        for grp in range(n_groups):
            lo = grp * PG
            hi = min(lo + PG, npair)
            if lo < npair:
                gbase[grp] = pair_batch[lo]
                span = int(pair_batch[hi - 1] - pair_batch[lo] + 1)
                W_need = max(W_need, span)
        shards.append((n0, n1, b, pr, pair_batch, gbase, npair))
    if not ok or W_need > 32:
        return _host_reference(node_embedding, W1, b1, W2, b2, W3, b3,
                               w_l2, batch, natoms_in)
    W = 16 if W_need <= 16 else 32

    W1hi = W1.astype(WIRE8).astype(np.float32)
    W1lo = (W1 - W1hi).astype(WIRE8)
    w1hl = np.ascontiguousarray(
        np.stack([W1hi.astype(WIRE8).T, W1lo.T], axis=1)
        .reshape(P, 2 * P)).astype(WIRE8)
    w2t = np.ascontiguousarray(W2.T).astype(WIRE16)
    w3t = np.ascontiguousarray(W3.T).astype(WIRE16)
    b1c = np.ascontiguousarray(b1[:, None])
    b2c = np.ascontiguousarray(b2[:, None])

    # ---- l=2 stream: fold w, pre-sum node pairs and c pairs,
    # fb-quantize ----
    el2w = (node_embedding[:, 4:9, :] * w_l2[0][None, None, :]) \
        .reshape(N, 5, 64, 2).sum(-1).reshape(N, ML2)
    el2_all = np.zeros((N_CORES, n_ppad, ML2), np.float32)
    for c in range(N_CORES):
        n0, n1, b, pr, pair_batch, gbase, npair = shards[c]
        if npair == 0:
            continue
        seg = el2w[n0:n1]
        if (n1 - n0) % 2 == 0 and np.array_equal(b[0::2], b[1::2]):
            el2_all[c, :npair] = seg[0::2] + seg[1::2]
        else:
            np.add.at(el2_all[c], pr, seg)
    el2q = _fb_quant_groups(
        el2_all.reshape(N_CORES * n_groups, PG, ML2), PG) \
        .reshape(N_CORES, n_ppad, ML2)

    in_maps = []
    for c in range(N_CORES):
        n0, n1, b, pr, pair_batch, gbase, npair = shards[c]
        nreal = n1 - n0
        x0T = np.zeros((P, n_pad), WIRE8)
        x0T[:, :nreal] = node_embedding[n0:n1, 0, :].T.astype(WIRE8)
        # pair-row = grp*PG + kt*P + p
        el2 = np.ascontiguousarray(
            el2q[c].reshape(n_groups, KT, P, ML2)
            .transpose(0, 2, 1, 3).reshape(n_groups, P, KT * ML2))
        lg = np.full(n_ppad, -1.0, np.float32)
        if npair:
            lg[:npair] = (pair_batch
                          - np.repeat(gbase, PG)[:npair]).astype(np.float32)
        A = (lg.reshape(n_groups, KT, P)[..., None]
             == np.arange(W, dtype=np.float32)).astype(WIRE8)
        A = np.ascontiguousarray(
            A.transpose(2, 0, 1, 3).reshape(P, n_groups * KT * W))
        in_maps.append({
            "x0T": x0T, "embL2": el2, "A_in": A,
            "w1hl": w1hl, "w2t": w2t, "w3t": w3t, "b1c": b1c, "b2c": b2c,
        })

    # all cores share shapes for the common layouts; build on core 0's
    nreal0 = shards[0][1] - shards[0][0]
    npair0 = shards[0][6]
    same = all((s[1] - s[0], s[6]) == (nreal0, npair0) for s in shards)
    if not same:
        # distinct per-core shapes: build with max bounds (padded inputs
        # make the extra work read zeros)
        nreal0 = max(s[1] - s[0] for s in shards)
        npair0 = max(s[6] for s in shards)
    nc = _build(n_groups, W, nreal0, npair0)

    res = bass_utils.run_bass_kernel_spmd(nc, in_maps,
                                          core_ids=list(range(N_CORES)))

    # ---- host epilogue ----
    inv = (1.0 / natoms_in.astype(np.float32)).astype(np.float32)
    n_ob = (n_groups + OB - 1) // OB
    node_scalar = np.empty(N, np.float32)
    Afull = np.zeros((G + 64, 5), np.float32)
    for c in range(N_CORES):
        n0, n1, b, pr, pair_batch, gbase, npair = shards[c]
        nreal = n1 - n0
        if nreal == 0:
            continue
        # scal layout: [n_ob, 4(chunk), OB, 256]; node = g*NG + c*256 + j
        sc = res.results[c]["scal"].reshape(n_ob, 4, OB, 256) \
            .transpose(0, 2, 1, 3).reshape(-1)[:nreal]
        node_scalar[n0:n1] = sc
        # S layout: [n_ob, 128, OB*5]; k-tile t -> rows 32t..32t+W
        Sc = res.results[c]["S_out"]
        for grp in range(n_groups):
            if grp * PG >= npair:
                continue
            gb = int(gbase[grp])
            j = grp % OB
            blk = Sc[grp // OB][:, j * 5:(j + 1) * 5]    # (128, 5)
            live = min(PG, npair - grp * PG)
            Kl = (live + P - 1) // P
            av = np.zeros((W, 5), np.float32)
            for t in range(min(Kl, 4)):
                av += blk[32 * t: 32 * t + W]
            Afull[gb:gb + W] += av
    iso = np.bincount(batch, weights=node_scalar + b3[0], minlength=G)
    iso = iso.astype(np.float32) * inv
    aniso = Afull[:G] * inv[:, None]
    dec = np.concatenate([iso[:, None], np.zeros((G, 3), np.float32), aniso],
                         axis=1)
    return (dec @ _CG).reshape(-1, 3, 3).astype(np.float32)
